# revision 41
# baseline (speedup 1.0000x reference)
"""LSEP loss kernel for Trainium2 (8 NeuronCores, data-parallel on batch).

loss = log1p( sum_b [ (sum_{c: t=0} e^{x_bc}) * (sum_{c: t=1} e^{-x_bc}) ] ) / B

Encoding: host ships x as bf16 and the target as m16 = t<<15 (uint16, the
bf16 sign-bit position). On device, z = x XOR m16 computes x*(1-2t), so ONE
exp pass yields e^x for negatives and e^{-x} for positives. With
S = sum_c e^z and D = sum_c (e^z XOR m16) = neg_sum - pos_sum, the per-row
product is 4*neg*pos = (S+D)(S-D).

Row sums: pair-halving tensor_tensor adds (the 2x DVE mode; tensor_reduce
and scalar_tensor_tensor both measured 1x on HW, and flat contiguous APs —
multi-dim strided views cost ~15%) down to 6 wide, then one 1x
tensor_reduce to f32 row scalars; ACT squares them with accum_out so the
per-tile sums of S^2/D^2 cost DVE nothing. Everything runs on DVE+ACT:
bitwise ops are DVE-only, and offloading tree stages to GPSIMD measured
slower (shared SBUF port, optimistic Pool cost model in the scheduler).
Engines execute their queues in order, so the loop is software-pipelined:
each tile's ACT-dependent ops (se-xor, trees) are emitted one tile after
its DMA/z-xor/exp. x and m16 ship interleaved in one [rows, 48] uint16
tensor (one DMA per tile).

Output: [128,2] per-core partials (sum of S^2, sum of D^2); host computes
(sum_S2 - sum_D2)/4 and applies log1p (the gather/unshard step).
"""

import numpy as np

B = 2_000_000
C = 24
NCORES = 8
P = 128
# half-size leading tiles shorten the pipeline fill ramp; steady-state
# tiles stay at the measured-optimal 196 rows per partition
KS = (98, 98) + (196,) * 9
TILES = len(KS)
RPC_RAW = B // NCORES            # 250_000 real rows per core
RPC = P * sum(KS)                # 250_880 padded rows per core

_cached = {}


def _build(rows, ks):
    from contextlib import ExitStack

    import concourse.bacc as bacc
    import concourse.tile as tile
    from concourse import mybir

    f32 = mybir.dt.float32
    bf16 = mybir.dt.bfloat16
    u16 = mybir.dt.uint16
    Alu = mybir.AluOpType
    Act = mybir.ActivationFunctionType
    X = mybir.AxisListType.X
    XY = mybir.AxisListType.XY

    nc = bacc.Bacc("TRN2", debug=False, num_devices=NCORES)
    xm = nc.dram_tensor("xm", [rows, 2 * C], u16, kind="ExternalInput").ap()
    out = nc.dram_tensor("o", [P, 2], f32, kind="ExternalOutput").ap()

    tiles = len(ks)
    xmv = []
    base = 0
    for ki in ks:
        xmv.append(
            xm[base : base + P * ki, :].rearrange("(p k) c -> p k c", p=P, k=ki)
        )
        base += P * ki

    with tile.TileContext(nc) as tc, ExitStack() as ctx:
        io = ctx.enter_context(tc.tile_pool(name="io", bufs=4))
        ep = ctx.enter_context(tc.tile_pool(name="ep", bufs=4))
        tp = ctx.enter_context(tc.tile_pool(name="tp", bufs=4))
        accp = ctx.enter_context(tc.tile_pool(name="accp", bufs=1))
        accS = accp.tile([P, tiles], f32)  # per-tile sum of S^2 rows
        accD = accp.tile([P, tiles], f32)  # per-tile sum of D^2 rows
        V = nc.vector
        G = nc.gpsimd

        st = {}  # per-tile tiles carried across pipeline stages

        def stage_a(i):
            # one DMA for interleaved [x | m16] rows; z = x ^ m; exp on ACT
            ki = ks[i]
            xmt = io.tile([P, ki, 2 * C], u16, tag="xm")
            nc.sync.dma_start(out=xmt, in_=xmv[i])
            xu = xmt[:, :, 0:C]
            mt = xmt[:, :, C : 2 * C]
            V.tensor_tensor(out=xu, in0=xu, in1=mt, op=Alu.bitwise_xor)
            e = ep.tile([P, ki, C], bf16, tag="e")
            nc.scalar.activation(out=e, in_=xu.bitcast(bf16), func=Act.Exp)
            st[i] = {"e": e, "m": mt}

        def stage_b(i):
            # se = e ^ m (in place over m); both trees on DVE
            ki = ks[i]
            e, mt = st[i]["e"], st[i]["m"]
            se = mt.bitcast(bf16)
            V.tensor_tensor(out=mt, in0=e.bitcast(u16), in1=mt,
                            op=Alu.bitwise_xor)
            s1 = tp.tile([P, ki, 12], bf16, tag="s1")
            V.tensor_add(s1, e[:, :, 0:12], e[:, :, 12:24])
            s2 = tp.tile([P, ki, 6], bf16, tag="s2")
            V.tensor_add(s2, s1[:, :, 0:6], s1[:, :, 6:12])
            sS = tp.tile([P, ki], f32, tag="sS")
            V.tensor_reduce(out=sS, in_=s2, axis=X, op=Alu.add)
            sq = tp.tile([P, ki], f32, tag="sq")
            nc.scalar.activation(out=sq, in_=sS, func=Act.Square,
                                 accum_out=accS[:, i : i + 1])
            d1 = tp.tile([P, ki, 12], bf16, tag="d1")
            V.tensor_add(d1, se[:, :, 0:12], se[:, :, 12:24])
            d2 = tp.tile([P, ki, 6], bf16, tag="d2")
            V.tensor_add(d2, d1[:, :, 0:6], d1[:, :, 6:12])
            dS = tp.tile([P, ki], f32, tag="dS")
            V.tensor_reduce(out=dS, in_=d2, axis=X, op=Alu.add)
            dq = tp.tile([P, ki], f32, tag="dq")
            nc.scalar.activation(out=dq, in_=dS, func=Act.Square,
                                 accum_out=accD[:, i : i + 1])
            del st[i]

        stage_a(0)
        stage_a(1)
        stage_b(0)
        for i in range(tiles):
            if i + 2 < tiles:
                stage_a(i + 2)
            if i + 1 < tiles:
                stage_b(i + 1)
        a1 = accp.tile([P, 2], f32)
        nc.vector.tensor_reduce(out=a1[:, 0:1], in_=accS, axis=X, op=Alu.add)
        nc.vector.tensor_reduce(out=a1[:, 1:2], in_=accD, axis=X, op=Alu.add)
        nc.sync.dma_start(out=out, in_=a1)
    nc.compile()
    return nc


def _get_nc():
    key = (RPC, KS)
    if key not in _cached:
        _cached[key] = _build(RPC, KS)
    return _cached[key]


def _f32_to_bf16_u16(a):
    # round-to-nearest-even f32 -> bf16, as uint16 bit pattern
    u = a.view(np.uint32)
    r = ((u >> 16) & 1) + np.uint32(0x7FFF)
    return ((u + r) >> 16).astype(np.uint16)


def _shard(input, target):
    xb = _f32_to_bf16_u16(input)
    mb = (target << 15).astype(np.uint16)
    in_maps = []
    for c in range(NCORES):
        xs = np.zeros((RPC, 2 * C), np.uint16)
        xs[:RPC_RAW, 0:C] = xb[c * RPC_RAW : (c + 1) * RPC_RAW]
        xs[:RPC_RAW, C : 2 * C] = mb[c * RPC_RAW : (c + 1) * RPC_RAW]
        in_maps.append({"xm": xs})
    return in_maps


_last_results = None


def kernel(input, target):
    global _last_results
    input = np.ascontiguousarray(np.asarray(input, dtype=np.float32))
    target = np.ascontiguousarray(np.asarray(target, dtype=np.int32))
    assert input.shape == (B, C) and target.shape == (B, C)

    from concourse.bass_utils import run_bass_kernel_spmd

    nc = _get_nc()
    in_maps = _shard(input, target)
    res = run_bass_kernel_spmd(nc, in_maps, core_ids=list(range(NCORES)))
    _last_results = res
    ssum = float(np.sum([r["o"][:, 0] for r in res.results], dtype=np.float64))
    dsum = float(np.sum([r["o"][:, 1] for r in res.results], dtype=np.float64))
    total = (ssum - dsum) / 4.0
    return np.asarray(np.log1p(total) / B, dtype=np.float32)


# revision 42
# speedup vs baseline: 1.1623x; 1.1623x over previous
"""LSEP loss kernel for Trainium2 (8 NeuronCores, data-parallel on batch).

loss = log1p( sum_b [ (sum_{c: t=0} e^{x_bc}) * (sum_{c: t=1} e^{-x_bc}) ] ) / B

Encoding: host ships x as bf16 and the target as m16 = t<<15 (uint16, the
bf16 sign-bit position). On device, z = x XOR m16 computes x*(1-2t), so ONE
exp pass yields e^x for negatives and e^{-x} for positives. With
S = sum_c e^z and D = sum_c (e^z XOR m16) = neg_sum - pos_sum, the per-row
product is 4*neg*pos = (S+D)(S-D).

Row sums: pair-halving tensor_tensor adds (the 2x DVE mode; tensor_reduce
and scalar_tensor_tensor both measured 1x on HW, and flat contiguous APs —
multi-dim strided views cost ~15%) down to 6 wide, then one 1x
tensor_reduce to f32 row scalars; ACT squares them with accum_out so the
per-tile sums of S^2/D^2 cost DVE nothing. Everything runs on DVE+ACT:
bitwise ops are DVE-only, and offloading tree stages to GPSIMD measured
slower (shared SBUF port, optimistic Pool cost model in the scheduler).
Engines execute their queues in order, so the loop is software-pipelined:
each tile's ACT-dependent ops (se-xor, trees) are emitted one tile after
its DMA/z-xor/exp. x and m16 ship interleaved in one [rows, 48] uint16
tensor (one DMA per tile).

Output: [128,2] per-core partials (sum of S^2, sum of D^2); host computes
(sum_S2 - sum_D2)/4 and applies log1p (the gather/unshard step).
"""

import numpy as np

B = 2_000_000
C = 24
NCORES = 8
P = 128
K = 196
TILES = 10
RPC_RAW = B // NCORES            # 250_000 real rows per core
RPC = P * K * TILES              # 250_880 padded rows per core

_cached = {}


def _build(rows, k, tiles):
    from contextlib import ExitStack

    import concourse.bacc as bacc
    import concourse.tile as tile
    from concourse import mybir

    f32 = mybir.dt.float32
    bf16 = mybir.dt.bfloat16
    u16 = mybir.dt.uint16
    Alu = mybir.AluOpType
    Act = mybir.ActivationFunctionType
    X = mybir.AxisListType.X
    XY = mybir.AxisListType.XY

    nc = bacc.Bacc("TRN2", debug=False, num_devices=NCORES)
    xm = nc.dram_tensor("xm", [rows, 2 * C], u16, kind="ExternalInput").ap()
    out = nc.dram_tensor("o", [P, 2], f32, kind="ExternalOutput").ap()

    xmv = xm.rearrange("(i p k) c -> i p k c", p=P, k=k)

    with tile.TileContext(nc) as tc, ExitStack() as ctx:
        io = ctx.enter_context(tc.tile_pool(name="io", bufs=4))
        ep = ctx.enter_context(tc.tile_pool(name="ep", bufs=4))
        tp = ctx.enter_context(tc.tile_pool(name="tp", bufs=4))
        accp = ctx.enter_context(tc.tile_pool(name="accp", bufs=1))
        accS = accp.tile([P, tiles], f32)  # per-tile sum of S^2 rows
        accD = accp.tile([P, tiles], f32)  # per-tile sum of D^2 rows
        V = nc.vector
        G = nc.gpsimd

        st = {}  # per-tile tiles carried across pipeline stages

        def stage_a(i):
            # one DMA for interleaved [x | m16] rows; z = x ^ m; exp on ACT
            xmt = io.tile([P, k, 2 * C], u16, tag="xm")
            nc.sync.dma_start(out=xmt, in_=xmv[i])
            xu = xmt[:, :, 0:C]
            mt = xmt[:, :, C : 2 * C]
            V.tensor_tensor(out=xu, in0=xu, in1=mt, op=Alu.bitwise_xor)
            e = ep.tile([P, k, C], bf16, tag="e")
            nc.scalar.activation(out=e, in_=xu.bitcast(bf16), func=Act.Exp)
            st[i] = {"e": e, "m": mt}

        def stage_b(i):
            # se = e ^ m (in place over m); S-tree on DVE, D-tree on GPSIMD
            e, mt = st[i]["e"], st[i]["m"]
            se = mt.bitcast(bf16)
            V.tensor_tensor(out=mt, in0=e.bitcast(u16), in1=mt,
                            op=Alu.bitwise_xor)
            s1 = tp.tile([P, k, 12], bf16, tag="s1")
            V.tensor_add(s1, e[:, :, 0:12], e[:, :, 12:24])
            s2 = tp.tile([P, k, 6], bf16, tag="s2")
            V.tensor_add(s2, s1[:, :, 0:6], s1[:, :, 6:12])
            sS = tp.tile([P, k], f32, tag="sS")
            V.tensor_reduce(out=sS, in_=s2, axis=X, op=Alu.add)
            sq = tp.tile([P, k], f32, tag="sq")
            nc.scalar.activation(out=sq, in_=sS, func=Act.Square,
                                 accum_out=accS[:, i : i + 1])
            d1 = tp.tile([P, k, 12], bf16, tag="d1")
            V.tensor_add(d1, se[:, :, 0:12], se[:, :, 12:24])
            d2 = tp.tile([P, k, 6], bf16, tag="d2")
            V.tensor_add(d2, d1[:, :, 0:6], d1[:, :, 6:12])
            dS = tp.tile([P, k], f32, tag="dS")
            V.tensor_reduce(out=dS, in_=d2, axis=X, op=Alu.add)
            dq = tp.tile([P, k], f32, tag="dq")
            nc.scalar.activation(out=dq, in_=dS, func=Act.Square,
                                 accum_out=accD[:, i : i + 1])
            del st[i]

        stage_a(0)
        stage_a(1)
        stage_b(0)
        for i in range(tiles):
            if i + 2 < tiles:
                stage_a(i + 2)
            if i + 1 < tiles:
                stage_b(i + 1)
        a1 = accp.tile([P, 2], f32)
        nc.vector.tensor_reduce(out=a1[:, 0:1], in_=accS, axis=X, op=Alu.add)
        nc.vector.tensor_reduce(out=a1[:, 1:2], in_=accD, axis=X, op=Alu.add)
        nc.sync.dma_start(out=out, in_=a1)
    nc.compile()
    return nc


def _get_nc():
    key = (RPC, K, TILES)
    if key not in _cached:
        _cached[key] = _build(RPC, K, TILES)
    return _cached[key]


def _f32_to_bf16_u16(a):
    # round-to-nearest-even f32 -> bf16, as uint16 bit pattern
    u = a.view(np.uint32)
    r = ((u >> 16) & 1) + np.uint32(0x7FFF)
    return ((u + r) >> 16).astype(np.uint16)


def _shard(input, target):
    xb = _f32_to_bf16_u16(input)
    mb = (target << 15).astype(np.uint16)
    in_maps = []
    for c in range(NCORES):
        xs = np.zeros((RPC, 2 * C), np.uint16)
        xs[:RPC_RAW, 0:C] = xb[c * RPC_RAW : (c + 1) * RPC_RAW]
        xs[:RPC_RAW, C : 2 * C] = mb[c * RPC_RAW : (c + 1) * RPC_RAW]
        in_maps.append({"xm": xs})
    return in_maps


_last_results = None


def kernel(input, target):
    global _last_results
    input = np.ascontiguousarray(np.asarray(input, dtype=np.float32))
    target = np.ascontiguousarray(np.asarray(target, dtype=np.int32))
    assert input.shape == (B, C) and target.shape == (B, C)

    from concourse.bass_utils import run_bass_kernel_spmd

    nc = _get_nc()
    in_maps = _shard(input, target)
    res = run_bass_kernel_spmd(nc, in_maps, core_ids=list(range(NCORES)))
    _last_results = res
    ssum = float(np.sum([r["o"][:, 0] for r in res.results], dtype=np.float64))
    dsum = float(np.sum([r["o"][:, 1] for r in res.results], dtype=np.float64))
    total = (ssum - dsum) / 4.0
    return np.asarray(np.log1p(total) / B, dtype=np.float32)


# revision 43
# speedup vs baseline: 1.1845x; 1.0192x over previous
"""LSEP loss kernel for Trainium2 (8 NeuronCores, data-parallel on batch).

loss = log1p( sum_b [ (sum_{c: t=0} e^{x_bc}) * (sum_{c: t=1} e^{-x_bc}) ] ) / B

Encoding: host ships x as bf16 and the target as m16 = t<<15 (uint16, the
bf16 sign-bit position). On device, z = x XOR m16 computes x*(1-2t), so ONE
exp pass yields e^x for negatives and e^{-x} for positives. With
S = sum_c e^z and D = sum_c (e^z XOR m16) = neg_sum - pos_sum, the per-row
product is 4*neg*pos = (S+D)(S-D).

Row sums: pair-halving tensor_tensor adds (the 2x DVE mode; tensor_reduce
and scalar_tensor_tensor both measured 1x on HW, and flat contiguous APs —
multi-dim strided views cost ~15%) down to 6 wide, then one 1x
tensor_reduce to f32 row scalars; ACT squares them with accum_out so the
per-tile sums of S^2/D^2 cost DVE nothing. Everything runs on DVE+ACT:
bitwise ops are DVE-only, and offloading tree stages to GPSIMD measured
slower (shared SBUF port, optimistic Pool cost model in the scheduler).
Engines execute their queues in order, so the loop is software-pipelined:
each tile's ACT-dependent ops (se-xor, trees) are emitted one tile after
its DMA/z-xor/exp. x and m16 ship interleaved in one [rows, 48] uint16
tensor (one DMA per tile).

Output: [128,2] per-core partials (sum of S^2, sum of D^2); host computes
(sum_S2 - sum_D2)/4 and applies log1p (the gather/unshard step).
"""

import numpy as np

B = 2_000_000
C = 24
NCORES = 8
P = 128
# half-size leading tiles shorten the pipeline fill ramp; steady-state
# tiles stay at the measured-optimal 196 rows per partition
KS = (98, 98) + (196,) * 9
TILES = len(KS)
RPC_RAW = B // NCORES            # 250_000 real rows per core
RPC = P * sum(KS)                # 250_880 padded rows per core

_cached = {}


def _build(rows, ks):
    from contextlib import ExitStack

    import concourse.bacc as bacc
    import concourse.tile as tile
    from concourse import mybir

    f32 = mybir.dt.float32
    bf16 = mybir.dt.bfloat16
    u16 = mybir.dt.uint16
    Alu = mybir.AluOpType
    Act = mybir.ActivationFunctionType
    X = mybir.AxisListType.X
    XY = mybir.AxisListType.XY

    nc = bacc.Bacc("TRN2", debug=False, num_devices=NCORES)
    xm = nc.dram_tensor("xm", [rows, 2 * C], u16, kind="ExternalInput").ap()
    out = nc.dram_tensor("o", [P, 2], f32, kind="ExternalOutput").ap()

    tiles = len(ks)
    xmv = []
    base = 0
    for ki in ks:
        xmv.append(
            xm[base : base + P * ki, :].rearrange("(p k) c -> p k c", p=P, k=ki)
        )
        base += P * ki

    with tile.TileContext(nc) as tc, ExitStack() as ctx:
        io = ctx.enter_context(tc.tile_pool(name="io", bufs=4))
        ep = ctx.enter_context(tc.tile_pool(name="ep", bufs=4))
        tp = ctx.enter_context(tc.tile_pool(name="tp", bufs=4))
        accp = ctx.enter_context(tc.tile_pool(name="accp", bufs=1))
        accS = accp.tile([P, tiles], f32)  # per-tile sum of S^2 rows
        accD = accp.tile([P, tiles], f32)  # per-tile sum of D^2 rows
        V = nc.vector
        G = nc.gpsimd

        st = {}  # per-tile tiles carried across pipeline stages

        def stage_a(i):
            # one DMA for interleaved [x | m16] rows; z = x ^ m; exp on ACT
            ki = ks[i]
            xmt = io.tile([P, ki, 2 * C], u16, tag="xm")
            nc.sync.dma_start(out=xmt, in_=xmv[i])
            xu = xmt[:, :, 0:C]
            mt = xmt[:, :, C : 2 * C]
            V.tensor_tensor(out=xu, in0=xu, in1=mt, op=Alu.bitwise_xor)
            e = ep.tile([P, ki, C], bf16, tag="e")
            nc.scalar.activation(out=e, in_=xu.bitcast(bf16), func=Act.Exp)
            st[i] = {"e": e, "m": mt}

        def stage_b(i):
            # se = e ^ m (in place over m); both trees on DVE
            ki = ks[i]
            e, mt = st[i]["e"], st[i]["m"]
            se = mt.bitcast(bf16)
            V.tensor_tensor(out=mt, in0=e.bitcast(u16), in1=mt,
                            op=Alu.bitwise_xor)
            s1 = tp.tile([P, ki, 12], bf16, tag="s1")
            V.tensor_add(s1, e[:, :, 0:12], e[:, :, 12:24])
            s2 = tp.tile([P, ki, 6], bf16, tag="s2")
            V.tensor_add(s2, s1[:, :, 0:6], s1[:, :, 6:12])
            sS = tp.tile([P, ki], f32, tag="sS")
            V.tensor_reduce(out=sS, in_=s2, axis=X, op=Alu.add)
            sq = tp.tile([P, ki], f32, tag="sq")
            nc.scalar.activation(out=sq, in_=sS, func=Act.Square,
                                 accum_out=accS[:, i : i + 1])
            d1 = tp.tile([P, ki, 12], bf16, tag="d1")
            V.tensor_add(d1, se[:, :, 0:12], se[:, :, 12:24])
            d2 = tp.tile([P, ki, 6], bf16, tag="d2")
            V.tensor_add(d2, d1[:, :, 0:6], d1[:, :, 6:12])
            dS = tp.tile([P, ki], f32, tag="dS")
            V.tensor_reduce(out=dS, in_=d2, axis=X, op=Alu.add)
            dq = tp.tile([P, ki], f32, tag="dq")
            nc.scalar.activation(out=dq, in_=dS, func=Act.Square,
                                 accum_out=accD[:, i : i + 1])
            del st[i]

        stage_a(0)
        stage_a(1)
        stage_b(0)
        for i in range(tiles):
            if i + 2 < tiles:
                stage_a(i + 2)
            if i + 1 < tiles:
                stage_b(i + 1)
        a1 = accp.tile([P, 2], f32)
        nc.vector.tensor_reduce(out=a1[:, 0:1], in_=accS, axis=X, op=Alu.add)
        nc.vector.tensor_reduce(out=a1[:, 1:2], in_=accD, axis=X, op=Alu.add)
        nc.sync.dma_start(out=out, in_=a1)
    nc.compile()
    return nc


def _get_nc():
    key = (RPC, KS)
    if key not in _cached:
        _cached[key] = _build(RPC, KS)
    return _cached[key]


def _f32_to_bf16_u16(a):
    # round-to-nearest-even f32 -> bf16, as uint16 bit pattern
    u = a.view(np.uint32)
    r = ((u >> 16) & 1) + np.uint32(0x7FFF)
    return ((u + r) >> 16).astype(np.uint16)


def _shard(input, target):
    xb = _f32_to_bf16_u16(input)
    mb = (target << 15).astype(np.uint16)
    in_maps = []
    for c in range(NCORES):
        xs = np.zeros((RPC, 2 * C), np.uint16)
        xs[:RPC_RAW, 0:C] = xb[c * RPC_RAW : (c + 1) * RPC_RAW]
        xs[:RPC_RAW, C : 2 * C] = mb[c * RPC_RAW : (c + 1) * RPC_RAW]
        in_maps.append({"xm": xs})
    return in_maps


_last_results = None


def kernel(input, target):
    global _last_results
    input = np.ascontiguousarray(np.asarray(input, dtype=np.float32))
    target = np.ascontiguousarray(np.asarray(target, dtype=np.int32))
    assert input.shape == (B, C) and target.shape == (B, C)

    from concourse.bass_utils import run_bass_kernel_spmd

    nc = _get_nc()
    in_maps = _shard(input, target)
    res = run_bass_kernel_spmd(nc, in_maps, core_ids=list(range(NCORES)))
    _last_results = res
    ssum = float(np.sum([r["o"][:, 0] for r in res.results], dtype=np.float64))
    dsum = float(np.sum([r["o"][:, 1] for r in res.results], dtype=np.float64))
    total = (ssum - dsum) / 4.0
    return np.asarray(np.log1p(total) / B, dtype=np.float32)


# revision 46
# speedup vs baseline: 1.1909x; 1.0054x over previous
"""LSEP loss kernel for Trainium2 (8 NeuronCores, data-parallel on batch).

loss = log1p( sum_b [ (sum_{c: t=0} e^{x_bc}) * (sum_{c: t=1} e^{-x_bc}) ] ) / B

Encoding: host ships x as bf16 and the target as m16 = t<<15 (uint16, the
bf16 sign-bit position). On device, z = x XOR m16 computes x*(1-2t), so ONE
exp pass yields e^x for negatives and e^{-x} for positives. With
S = sum_c e^z and D = sum_c (e^z XOR m16) = neg_sum - pos_sum, the per-row
product is 4*neg*pos = (S+D)(S-D).

Row sums: pair-halving tensor_tensor adds (the 2x DVE mode; tensor_reduce
and scalar_tensor_tensor both measured 1x on HW, and flat contiguous APs —
multi-dim strided views cost ~15%) down to 6 wide, then one 1x
tensor_reduce to f32 row scalars; ACT squares them with accum_out so the
per-tile sums of S^2/D^2 cost DVE nothing. Everything runs on DVE+ACT:
bitwise ops are DVE-only, and offloading tree stages to GPSIMD measured
slower (shared SBUF port, optimistic Pool cost model in the scheduler).
Engines execute their queues in order, so the loop is software-pipelined:
each tile's ACT-dependent ops (se-xor, trees) are emitted one tile after
its DMA/z-xor/exp. x and m16 ship interleaved in one [rows, 48] uint16
tensor (one DMA per tile).

Output: [128,2] per-core partials (sum of S^2, sum of D^2); host computes
(sum_S2 - sum_D2)/4 and applies log1p (the gather/unshard step).
"""

import numpy as np

B = 2_000_000
C = 24
NCORES = 8
P = 128
# half-size leading tiles shorten the pipeline fill ramp; steady-state
# tiles stay at the measured-optimal 196 rows per partition
KS = (98, 98) + (196,) * 9
TILES = len(KS)
RPC_RAW = B // NCORES            # 250_000 real rows per core
RPC = P * sum(KS)                # 250_880 padded rows per core

_cached = {}


def _build(rows, ks):
    from contextlib import ExitStack

    import concourse.bacc as bacc
    import concourse.tile as tile
    from concourse import mybir

    f32 = mybir.dt.float32
    bf16 = mybir.dt.bfloat16
    u16 = mybir.dt.uint16
    Alu = mybir.AluOpType
    Act = mybir.ActivationFunctionType
    X = mybir.AxisListType.X
    XY = mybir.AxisListType.XY

    nc = bacc.Bacc("TRN2", debug=False, num_devices=NCORES)
    xm = nc.dram_tensor("xm", [rows, 2 * C], u16, kind="ExternalInput").ap()
    out = nc.dram_tensor("o", [P, 2], f32, kind="ExternalOutput").ap()

    tiles = len(ks)
    xmv = []
    base = 0
    for ki in ks:
        xmv.append(
            xm[base : base + P * ki, :].rearrange("(p k) c -> p k c", p=P, k=ki)
        )
        base += P * ki

    with tile.TileContext(nc) as tc, ExitStack() as ctx:
        io = ctx.enter_context(tc.tile_pool(name="io", bufs=4))
        ep = ctx.enter_context(tc.tile_pool(name="ep", bufs=4))
        tp = ctx.enter_context(tc.tile_pool(name="tp", bufs=3))
        hp = ctx.enter_context(tc.tile_pool(name="hp", bufs=2))
        accp = ctx.enter_context(tc.tile_pool(name="accp", bufs=1))
        # one extra column: the last tile's stage_b runs as two k-halves
        # (shorter drain chain), each with its own accumulator slot
        accS = accp.tile([P, tiles + 1], f32)  # per-tile sum of S^2 rows
        accD = accp.tile([P, tiles + 1], f32)  # per-tile sum of D^2 rows
        V = nc.vector
        G = nc.gpsimd

        st = {}  # per-tile tiles carried across pipeline stages

        def stage_a(i):
            # one DMA for interleaved [x | m16] rows; z = x ^ m; exp on ACT
            ki = ks[i]
            xmt = io.tile([P, ki, 2 * C], u16, tag="xm")
            nc.sync.dma_start(out=xmt, in_=xmv[i])
            xu = xmt[:, :, 0:C]
            mt = xmt[:, :, C : 2 * C]
            V.tensor_tensor(out=xu, in0=xu, in1=mt, op=Alu.bitwise_xor)
            e = ep.tile([P, ki, C], bf16, tag="e")
            nc.scalar.activation(out=e, in_=xu.bitcast(bf16), func=Act.Exp)
            st[i] = {"e": e, "m": mt}

        def stage_b(i):
            # se = e ^ m (in place over m); both trees on DVE
            ki = ks[i]
            e, mt = st[i]["e"], st[i]["m"]
            se = mt.bitcast(bf16)
            V.tensor_tensor(out=mt, in0=e.bitcast(u16), in1=mt,
                            op=Alu.bitwise_xor)
            s1 = tp.tile([P, ki, 12], bf16, tag="s1")
            V.tensor_add(s1, e[:, :, 0:12], e[:, :, 12:24])
            s2 = tp.tile([P, ki, 6], bf16, tag="s2")
            V.tensor_add(s2, s1[:, :, 0:6], s1[:, :, 6:12])
            sS = tp.tile([P, ki], f32, tag="sS")
            V.tensor_reduce(out=sS, in_=s2, axis=X, op=Alu.add)
            sq = tp.tile([P, ki], f32, tag="sq")
            nc.scalar.activation(out=sq, in_=sS, func=Act.Square,
                                 accum_out=accS[:, i : i + 1])
            d1 = tp.tile([P, ki, 12], bf16, tag="d1")
            V.tensor_add(d1, se[:, :, 0:12], se[:, :, 12:24])
            d2 = tp.tile([P, ki, 6], bf16, tag="d2")
            V.tensor_add(d2, d1[:, :, 0:6], d1[:, :, 6:12])
            dS = tp.tile([P, ki], f32, tag="dS")
            V.tensor_reduce(out=dS, in_=d2, axis=X, op=Alu.add)
            dq = tp.tile([P, ki], f32, tag="dq")
            nc.scalar.activation(out=dq, in_=dS, func=Act.Square,
                                 accum_out=accD[:, i : i + 1])
            del st[i]

        def stage_b_half(i, k0, k1, col):
            # one k-slice of stage_b for the last tile; the two slices'
            # dependency chains overlap, shortening the pipeline drain
            kh = k1 - k0
            e = st[i]["e"][:, k0:k1]
            mt = st[i]["m"][:, k0:k1]
            se = mt.bitcast(bf16)
            V.tensor_tensor(out=mt, in0=e.bitcast(u16), in1=mt,
                            op=Alu.bitwise_xor)
            s1 = hp.tile([P, kh, 12], bf16, tag="s1h")
            V.tensor_add(s1, e[:, :, 0:12], e[:, :, 12:24])
            s2 = hp.tile([P, kh, 6], bf16, tag="s2h")
            V.tensor_add(s2, s1[:, :, 0:6], s1[:, :, 6:12])
            sS = hp.tile([P, kh], f32, tag="sSh")
            V.tensor_reduce(out=sS, in_=s2, axis=X, op=Alu.add)
            sq = hp.tile([P, kh], f32, tag="sqh")
            nc.scalar.activation(out=sq, in_=sS, func=Act.Square,
                                 accum_out=accS[:, col : col + 1])
            d1 = hp.tile([P, kh, 12], bf16, tag="d1h")
            V.tensor_add(d1, se[:, :, 0:12], se[:, :, 12:24])
            d2 = hp.tile([P, kh, 6], bf16, tag="d2h")
            V.tensor_add(d2, d1[:, :, 0:6], d1[:, :, 6:12])
            dS = hp.tile([P, kh], f32, tag="dSh")
            V.tensor_reduce(out=dS, in_=d2, axis=X, op=Alu.add)
            dq = hp.tile([P, kh], f32, tag="dqh")
            nc.scalar.activation(out=dq, in_=dS, func=Act.Square,
                                 accum_out=accD[:, col : col + 1])

        stage_a(0)
        stage_a(1)
        stage_b(0)
        last = tiles - 1
        for i in range(tiles):
            if i + 2 < tiles:
                stage_a(i + 2)
            if i + 1 < tiles:
                if i + 1 == last:
                    kl = ks[last]
                    stage_b_half(last, 0, kl // 2, last)
                    stage_b_half(last, kl // 2, kl, tiles)
                    del st[last]
                else:
                    stage_b(i + 1)
        a1 = accp.tile([P, 2], f32)
        nc.vector.tensor_reduce(out=a1[:, 0:1], in_=accS, axis=X, op=Alu.add)
        nc.vector.tensor_reduce(out=a1[:, 1:2], in_=accD, axis=X, op=Alu.add)
        nc.sync.dma_start(out=out, in_=a1)
    nc.compile()
    return nc


def _get_nc():
    key = (RPC, KS)
    if key not in _cached:
        _cached[key] = _build(RPC, KS)
    return _cached[key]


def _f32_to_bf16_u16(a):
    # round-to-nearest-even f32 -> bf16, as uint16 bit pattern
    u = a.view(np.uint32)
    r = ((u >> 16) & 1) + np.uint32(0x7FFF)
    return ((u + r) >> 16).astype(np.uint16)


def _shard(input, target):
    xb = _f32_to_bf16_u16(input)
    mb = (target << 15).astype(np.uint16)
    in_maps = []
    for c in range(NCORES):
        xs = np.zeros((RPC, 2 * C), np.uint16)
        xs[:RPC_RAW, 0:C] = xb[c * RPC_RAW : (c + 1) * RPC_RAW]
        xs[:RPC_RAW, C : 2 * C] = mb[c * RPC_RAW : (c + 1) * RPC_RAW]
        in_maps.append({"xm": xs})
    return in_maps


_last_results = None


def kernel(input, target):
    global _last_results
    input = np.ascontiguousarray(np.asarray(input, dtype=np.float32))
    target = np.ascontiguousarray(np.asarray(target, dtype=np.int32))
    assert input.shape == (B, C) and target.shape == (B, C)

    from concourse.bass_utils import run_bass_kernel_spmd

    nc = _get_nc()
    in_maps = _shard(input, target)
    res = run_bass_kernel_spmd(nc, in_maps, core_ids=list(range(NCORES)))
    _last_results = res
    ssum = float(np.sum([r["o"][:, 0] for r in res.results], dtype=np.float64))
    dsum = float(np.sum([r["o"][:, 1] for r in res.results], dtype=np.float64))
    total = (ssum - dsum) / 4.0
    return np.asarray(np.log1p(total) / B, dtype=np.float32)
